# revision 17
# baseline (speedup 1.0000x reference)
import numpy as np

# Per-head sparse MoE (top-2 of 8 experts), expert-parallel across 8 NeuronCores.
# B=8192 tokens, N=16 heads, D=128, H=384, E=8.
# Host: router (replicates reference jnp ops bit-exactly) + token dispatch.
# Device (core e = expert e): per head, h1=w1.T@x, g=wg.T@x, h=h1*silu(g),
# y=w2.T@h on capacity-padded token batches; the per-token routing weight is
# applied on the HOST during unpack (it commutes through the w2 contraction
# along the token axis), so the device never touches it.
#
# HW microbenchmarks (ns per [128,512] op, this session):
#   DVE mul  h1(PSUM f32) x sil(SBUF f32) -> SBUF bf16 : 658.5
#   DVE mul  all-SBUF bf16 (2x_1p)                     : 332.8
#   DVE copy PSUM f32 -> SBUF bf16                     : 651.3
#   ACT copy PSUM f32 -> SBUF bf16                     : 526.6
#   ACT silu [128,1024] PSUM -> SBUF f32               : 1020.3
#   Pool mul SBUF bf16                                 : 1065
#   PE matmul 512-col bf16 back-to-back (same bank)    : 266
# Engine balance (C=2112): PE ~131us, DVE muls ~130us, ACT silu ~105us;
# w2-output evacuation (66 512-tiles) split ~15% DVE / 85% ACT equalizes
# DVE/ACT at ~137us. sil stays f32 (bf16 sil measured slower for the PSUM
# mul, and path-B ACT-copy schemes lose the LP balance).
# Structure: w2 matmuls accumulate per-hblock into the pair's o-psum tiles
# as soon as hbt[hb] is ready (finer deps than batching all 3 hblocks).
# Dummy 1-wide silu up front overlaps the ~2.7us activation-table load.
# y-DMA descriptors issue from the idle GpSimd queue.
# x/weights/y travel bf16; accumulate f32 in PSUM.

B, N, D, H, E = 8192, 16, 128, 384, 8
N_CORES = 8
HB = H // 128   # h-blocks of 128
WCOL = 3 * H    # packed weight columns per head: [w1 | wg | w2r]

USE_BF16 = True
CAP_ALIGN = 64  # capacity padding granularity

_nc_cache = {}


def _build_bass(C, chunks=None, reps=1, evac_dve=(0, 7, 14), evac_period=20,
                sp_bufs=4, hp_bufs=2, split_x_dma=True, gW=1024,
                bg=2, bh=2, bo=2, ydma_gp=True, warm_act=True, pipe=True,
                ob_bufs=4, silu_split=False, xw_bufs=2, diag=None):
    import concourse.bacc as bacc
    import concourse.mybir as mybir
    import concourse.tile as tile

    f32 = mybir.dt.float32
    bf = mybir.dt.bfloat16
    SILU = mybir.ActivationFunctionType.Silu

    pairs = [(p0, min(gW, C - p0)) for p0 in range(0, C, gW)]

    def subs(pw):
        return [(s, min(512, pw - s)) for s in range(0, pw, 512)]

    nc = bacc.Bacc("TRN2", target_bir_lowering=False, debug=False,
                   num_devices=N_CORES)
    xT = nc.dram_tensor("xT", [N, 128, C], bf, kind="ExternalInput").ap()
    wall = nc.dram_tensor("wall", [N, 128, WCOL], bf, kind="ExternalInput").ap()
    yT = nc.dram_tensor("yT", [N, 128, C], bf, kind="ExternalOutput").ap()

    copy_cnt = [0]

    with tile.TileContext(nc) as tc:
        with tc.tile_pool(name="xp", bufs=xw_bufs) as xp, \
             tc.tile_pool(name="wp", bufs=xw_bufs) as wp, \
             tc.tile_pool(name="sp", bufs=sp_bufs) as sp, \
             tc.tile_pool(name="hp", bufs=hp_bufs) as hp, \
             tc.tile_pool(name="ob", bufs=ob_bufs) as ob, \
             tc.tile_pool(name="pg", bufs=bg, space="PSUM") as pg, \
             tc.tile_pool(name="ph", bufs=bh, space="PSUM") as ph, \
             tc.tile_pool(name="po", bufs=bo, space="PSUM") as po:

            # pending: previous pair awaiting staggered o emission.
            # [n, p0, pw, hbts, w_t, o_ps]; o_ps filled lazily at hb0.
            pending = [None]

            def o_mm(hb):
                # w2 contribution of hblock hb for the PREVIOUS pair, issued
                # during the current pair's hb slot: its mul finished a full
                # pair ago, so the PE's in-order stream never waits on DVE.
                pend = pending[0]
                if pend is None:
                    return
                n, p0, pw, hbts, w_t, o_ps = pend
                w2of = 2 * H
                if hb == 0:
                    for (s0, sw) in subs(pw):
                        o_ps[s0] = po.tile([128, 512], f32, tag="o",
                                           name=f"o{s0}")
                for (s0, sw) in subs(pw):
                    nc.tensor.matmul(
                        o_ps[s0][:, :sw],
                        w_t[:, w2of + hb * 128:w2of + (hb + 1) * 128],
                        hbts[hb][:, s0:s0 + sw],
                        start=(hb == 0), stop=(hb == HB - 1),
                    )

            def o_evac():
                pend = pending[0]
                if pend is None:
                    return
                n, p0, pw, hbts, w_t, o_ps = pend
                for (s0, sw) in subs(pw):
                    o_sb = ob.tile([128, 512], bf, tag="osb")
                    if copy_cnt[0] % evac_period in evac_dve:
                        nc.vector.tensor_copy(o_sb[:, :sw], o_ps[s0][:, :sw])
                    else:
                        nc.scalar.copy(o_sb[:, :sw], o_ps[s0][:, :sw])
                    copy_cnt[0] += 1
                    (nc.gpsimd if ydma_gp else nc.sync).dma_start(
                        yT[n][:, p0 + s0:p0 + s0 + sw], o_sb[:, :sw]
                    )
                pending[0] = None

            def flush_pending():
                for hb in range(HB):
                    o_mm(hb)
                o_evac()

            const_hbt = [None]

            def head_pe_only(n, x_t, w_t):
                # diagnostic: the exact PE instruction stream (g/h1/o matmuls
                # with the same weight-change pattern) but no ACT/DVE deps.
                w1of, wgof, w2of = 0, H, 2 * H
                hbt = const_hbt[0]
                for (p0, pw) in pairs:
                    o_ps = {}
                    for (s0, sw) in subs(pw):
                        o_ps[s0] = po.tile([128, 512], f32, tag="o",
                                           name=f"o{s0}")
                    for hb in range(HB):
                        g_t = pg.tile([128, gW], f32, tag="g")
                        for (s0, sw) in subs(pw):
                            nc.tensor.matmul(
                                g_t[:, s0:s0 + sw],
                                w_t[:, wgof + hb * 128:wgof + (hb + 1) * 128],
                                x_t[:, p0 + s0:p0 + s0 + sw],
                                start=True, stop=True,
                            )
                        for (s0, sw) in subs(pw):
                            h1_t = ph.tile([128, 512], f32, tag="h1")
                            nc.tensor.matmul(
                                h1_t[:, :sw],
                                w_t[:, w1of + hb * 128:w1of + (hb + 1) * 128],
                                x_t[:, p0 + s0:p0 + s0 + sw],
                                start=True, stop=True,
                            )
                        for (s0, sw) in subs(pw):
                            nc.tensor.matmul(
                                o_ps[s0][:, :sw],
                                w_t[:, w2of + hb * 128:w2of + (hb + 1) * 128],
                                hbt[:, s0:s0 + sw],
                                start=(hb == 0), stop=(hb == HB - 1),
                            )

            def head(n, x_t, w_t):
                if diag == "pe_only":
                    return head_pe_only(n, x_t, w_t)
                w1of, wgof = 0, H
                # head 0 starts with the small tail pair: its g-matmul only
                # needs a tiny x slice, so silu/mul start ~1.5us earlier.
                hpairs = (pairs[-1:] + pairs[:-1]) if n == 0 else pairs
                for (p0, pw) in hpairs:
                    hbts = []
                    for hb in range(HB):
                        g_t = pg.tile([128, gW], f32, tag="g")
                        for (s0, sw) in subs(pw):
                            nc.tensor.matmul(
                                g_t[:, s0:s0 + sw],
                                w_t[:, wgof + hb * 128:wgof + (hb + 1) * 128],
                                x_t[:, p0 + s0:p0 + s0 + sw],
                                start=True, stop=True,
                            )
                        sil = sp.tile([128, gW], f32, tag="sil")
                        if diag in ("slim_silu", "skeleton"):
                            # diagnostic: tiny silu keeps the dep, drops load
                            nc.vector.memset(sil[:, :pw], 1.0)
                            nc.scalar.activation(sil[:, :64], g_t[:, :64], SILU)
                        elif silu_split:
                            # per-512 silu: +~144ns/hb ACT but the first mul
                            # can start ~500ns earlier (finer ACT->DVE pipe)
                            for (s0, sw) in subs(pw):
                                nc.scalar.activation(sil[:, s0:s0 + sw],
                                                     g_t[:, s0:s0 + sw], SILU)
                        else:
                            nc.scalar.activation(sil[:, :pw], g_t[:, :pw], SILU)
                        hbt = hp.tile([128, gW], bf, tag=f"hbt{hb}")
                        if diag in ("slim_mul", "skeleton"):
                            nc.vector.memset(hbt[:, :pw], 1.0)
                        for (s0, sw) in subs(pw):
                            h1_t = ph.tile([128, 512], f32, tag="h1")
                            nc.tensor.matmul(
                                h1_t[:, :sw],
                                w_t[:, w1of + hb * 128:w1of + (hb + 1) * 128],
                                x_t[:, p0 + s0:p0 + s0 + sw],
                                start=True, stop=True,
                            )
                            if diag in ("slim_mul", "skeleton"):
                                if s0 == 0:
                                    nc.vector.tensor_mul(
                                        hbt[:, :64], h1_t[:, :64], sil[:, :64]
                                    )
                            else:
                                nc.vector.tensor_mul(
                                    hbt[:, s0:s0 + sw], h1_t[:, :sw],
                                    sil[:, s0:s0 + sw]
                                )
                        hbts.append(hbt)
                        if pipe:
                            o_mm(hb)
                    if pipe:
                        o_evac()
                        pending[0] = [n, p0, pw, hbts, w_t, {}]
                    else:
                        pending[0] = [n, p0, pw, hbts, w_t, {}]
                        flush_pending()

            def body():
                if warm_act:
                    warm = sp.tile([128, 1], f32, tag="warm", name="warm")
                    nc.vector.memset(warm[:], 0.0)
                    nc.scalar.activation(warm[:], warm[:], SILU)
                if diag == "pe_only" and const_hbt[0] is None:
                    cb = hp.tile([128, gW], bf, tag="chbt", name="chbt")
                    nc.vector.memset(cb[:], 0.5)
                    const_hbt[0] = cb
                for n in range(N):
                    # w first: the first g-matmul needs the weights; x can
                    # stream in behind it (head 0 consumes the tail slice
                    # first, so that chunk is DMA'd first).
                    w_t = wp.tile([128, WCOL], bf, tag="w")
                    nc.sync.dma_start(w_t[:], wall[n])
                    x_t = xp.tile([128, C], bf, tag="x")
                    if split_x_dma and n == 0:
                        for (p0, pw) in (pairs[-1:] + pairs[:-1]):
                            nc.sync.dma_start(x_t[:, p0:p0 + pw],
                                              xT[n][:, p0:p0 + pw])
                    else:
                        nc.sync.dma_start(x_t[:], xT[n])
                    head(n, x_t, w_t)
                if pipe and pending[0] is not None:
                    flush_pending()

            if reps == 1:
                body()
            else:
                with tc.For_i(0, reps, 1):
                    body()
    nc.finalize()
    return nc


def _route(x, router_w):
    import jax
    import jax.numpy as jnp

    router_logits = jnp.asarray(x).reshape(B, N * D) @ jnp.asarray(router_w).T
    topk_logits, topk_idx = jax.lax.top_k(router_logits, 2)
    topk_w = jax.nn.softmax(topk_logits, axis=-1)
    return np.asarray(topk_idx), np.asarray(topk_w).astype(np.float32)


def _dispatch(x, topk_idx, topk_w):
    idx_list, wgt_list = [], []
    for e in range(E):
        sel = np.nonzero((topk_idx == e).any(axis=1))[0]
        we = np.where(topk_idx[sel, 0] == e, topk_w[sel, 0], topk_w[sel, 1])
        idx_list.append(sel)
        wgt_list.append(we.astype(np.float32))
    maxL = max(max(len(s) for s in idx_list), 1)
    C = ((maxL + CAP_ALIGN - 1) // CAP_ALIGN) * CAP_ALIGN
    chunks = []
    c0 = 0
    while c0 < C:
        cw = 512 if C - c0 >= 512 else C - c0
        chunks.append((c0, cw))
        c0 += cw
    return idx_list, wgt_list, C, tuple(chunks)


def _make_in_maps(x, w1, w_gate, w2, idx_list, wgt_list, C, bf16=USE_BF16):
    if bf16:
        import ml_dtypes
        dt = ml_dtypes.bfloat16
    else:
        dt = np.float32
    in_maps = []
    xTfull = np.ascontiguousarray(x.transpose(1, 2, 0).astype(dt))  # (N,128,B)
    for e in range(E):
        sel = idx_list[e]
        L = len(sel)
        xg = np.zeros((N, 128, C), dt)
        if L:
            xg[:, :, :L] = xTfull[:, :, sel]
        w2r = w2[e].reshape(N, HB, 128, 128).transpose(0, 2, 1, 3).reshape(N, 128, H)
        wcat = np.ascontiguousarray(np.concatenate(
            [w1[e].astype(dt), w_gate[e].astype(dt), w2r.astype(dt)], axis=2
        ))  # (N,128,3H)
        in_maps.append({"xT": xg, "wall": wcat})
    return in_maps


_runner_cache = {}


def _make_runner(nc):
    """Cached jitted executor equivalent to bass2jax.run_bass_via_pjrt,
    avoiding per-call retrace/rejit of the shard_map wrapper."""
    import jax
    import concourse.mybir as mybir
    from concourse import bass2jax
    from jax.sharding import Mesh, PartitionSpec
    from jax.experimental.shard_map import shard_map

    bass2jax.install_neuronx_cc_hook()
    partition_name = nc.partition_id_tensor.name if nc.partition_id_tensor else None
    in_names, out_names, out_avals, out_shapes = [], [], [], []
    for alloc in nc.m.functions[0].allocations:
        if not isinstance(alloc, mybir.MemoryLocationSet):
            continue
        name = alloc.memorylocations[0].name
        if alloc.kind == "ExternalInput":
            if name != partition_name:
                in_names.append(name)
        elif alloc.kind == "ExternalOutput":
            shape = tuple(alloc.tensor_shape)
            dtype = mybir.dt.np(alloc.dtype)
            out_names.append(name)
            out_avals.append(jax.core.ShapedArray(shape, dtype))
            out_shapes.append((shape, dtype))
    all_in_names = list(in_names) + list(out_names)
    if partition_name is not None:
        all_in_names.append(partition_name)

    def _body(*args):
        operands = list(args)
        if partition_name is not None:
            operands.append(bass2jax.partition_id_tensor())
        return tuple(bass2jax._bass_exec_p.bind(
            *operands,
            out_avals=tuple(out_avals),
            in_names=tuple(all_in_names),
            out_names=tuple(out_names),
            lowering_input_output_aliases=(),
            sim_require_finite=True,
            sim_require_nnan=True,
            nc=nc,
        ))

    mesh = Mesh(np.asarray(jax.devices()[:N_CORES]), ("core",))
    nio = len(in_names) + len(out_names)
    sharded = jax.jit(
        shard_map(_body, mesh=mesh,
                  in_specs=(PartitionSpec("core"),) * nio,
                  out_specs=(PartitionSpec("core"),) * len(out_names),
                  check_rep=False),
        keep_unused=True,
    )

    def run(in_maps):
        concat_in = [
            np.concatenate([np.asarray(in_maps[c][nm]) for c in range(N_CORES)],
                           axis=0)
            for nm in in_names
        ]
        concat_zeros = [
            np.zeros((N_CORES * s[0], *s[1:]), d) for (s, d) in out_shapes
        ]
        outs = sharded(*(concat_in + concat_zeros))
        outs = [np.asarray(o) for o in outs]
        results = []
        for c in range(N_CORES):
            res = {}
            for (nm, o, (s, d)) in zip(out_names, outs, out_shapes):
                res[nm] = o[c * s[0]:(c + 1) * s[0]]
            results.append(res)
        return results

    return run


def kernel(**inputs):
    x = np.asarray(inputs["x"], dtype=np.float32)
    router_w = np.asarray(inputs["router_w"], dtype=np.float32)
    w1 = np.asarray(inputs["w1"], dtype=np.float32)
    w_gate = np.asarray(inputs["w_gate"], dtype=np.float32)
    w2 = np.asarray(inputs["w2"], dtype=np.float32)

    topk_idx, topk_w = _route(x, router_w)
    idx_list, wgt_list, C, chunks = _dispatch(x, topk_idx, topk_w)

    key = (C, chunks, 1, USE_BF16)
    if key not in _nc_cache:
        _nc_cache[key] = _build_bass(C, chunks)
    nc = _nc_cache[key]

    in_maps = _make_in_maps(x, w1, w_gate, w2, idx_list, wgt_list, C)

    if key not in _runner_cache:
        from concourse import bass_utils
        res = bass_utils.run_bass_kernel_spmd(
            nc, in_maps, core_ids=list(range(N_CORES)), trace=False
        )
        results = res.results
        _runner_cache[key] = _make_runner(nc)
    else:
        results = _runner_cache[key](in_maps)

    out = np.zeros((B, N, D), np.float32)
    for e in range(E):
        sel = idx_list[e]
        L = len(sel)
        if L:
            yT = np.asarray(results[e]["yT"], dtype=np.float32)  # (N,128,C)
            out[sel] += yT[:, :, :L].transpose(2, 0, 1) * \
                wgt_list[e][:, None, None]
    return out


# revision 19
# speedup vs baseline: 1.1905x; 1.1905x over previous
import numpy as np

# Per-head sparse MoE (top-2 of 8 experts), expert-parallel across 8 NeuronCores.
# B=8192 tokens, N=16 heads, D=128, H=384, E=8.
# Host: router (replicates reference jnp ops bit-exactly) + token dispatch.
# Device (core e = expert e): per head, h1=w1.T@x, g=wg.T@x, h=h1*silu(g),
# y=w2.T@h on capacity-padded token batches; the per-token routing weight is
# applied on the HOST during unpack (it commutes through the w2 contraction
# along the token axis), so the device never touches it.
#
# HW microbenchmarks (ns per [128,512] op, this session):
#   DVE mul  h1(PSUM f32) x sil(SBUF f32) -> SBUF bf16 : 658.5
#   DVE mul  all-SBUF bf16 (2x_1p)                     : 332.8
#   DVE copy PSUM f32 -> SBUF bf16                     : 651.3
#   ACT copy PSUM f32 -> SBUF bf16                     : 526.6
#   ACT silu [128,1024] PSUM -> SBUF f32               : 1020.3
#   Pool mul SBUF bf16                                 : 1065
#   PE matmul 512-col bf16 back-to-back (same bank)    : 266
# Engine balance (C=2112): PE ~131us, DVE muls ~130us, ACT silu ~105us;
# w2-output evacuation (66 512-tiles) split ~15% DVE / 85% ACT equalizes
# DVE/ACT at ~137us. sil stays f32 (bf16 sil measured slower for the PSUM
# mul, and path-B ACT-copy schemes lose the LP balance).
# Structure: w2 matmuls accumulate per-hblock into the pair's o-psum tiles
# as soon as hbt[hb] is ready (finer deps than batching all 3 hblocks).
# Dummy 1-wide silu up front overlaps the ~2.7us activation-table load.
# y-DMA descriptors issue from the idle GpSimd queue.
# x/weights/y travel bf16; accumulate f32 in PSUM.

B, N, D, H, E = 8192, 16, 128, 384, 8
N_CORES = 8
HB = H // 128   # h-blocks of 128
WCOL = 3 * H    # packed weight columns per head: [w1 | wg | w2r]

USE_BF16 = True
CAP_ALIGN = 64  # capacity padding granularity

_nc_cache = {}


def _build_bass(C, chunks=None, reps=1, evac_dve=(0, 7, 14), evac_period=20,
                sp_bufs=4, hp_bufs=2, split_x_dma=True, gW=1024,
                bg=2, bh=2, bo=2, ydma_gp=True, warm_act=True, pipe=True,
                ob_bufs=4, silu_split=False, xw_bufs=2, diag=None):
    import concourse.bacc as bacc
    import concourse.mybir as mybir
    import concourse.tile as tile

    f32 = mybir.dt.float32
    bf = mybir.dt.bfloat16
    SILU = mybir.ActivationFunctionType.Silu

    pairs = [(p0, min(gW, C - p0)) for p0 in range(0, C, gW)]

    def subs(pw):
        return [(s, min(512, pw - s)) for s in range(0, pw, 512)]

    nc = bacc.Bacc("TRN2", target_bir_lowering=False, debug=False,
                   num_devices=N_CORES)
    xT = nc.dram_tensor("xT", [N, 128, C], bf, kind="ExternalInput").ap()
    wall = nc.dram_tensor("wall", [N, 128, WCOL], bf, kind="ExternalInput").ap()
    yT = nc.dram_tensor("yT", [N, 128, C], bf, kind="ExternalOutput").ap()

    copy_cnt = [0]

    with tile.TileContext(nc) as tc:
        with tc.tile_pool(name="xp", bufs=xw_bufs) as xp, \
             tc.tile_pool(name="wp", bufs=xw_bufs) as wp, \
             tc.tile_pool(name="sp", bufs=sp_bufs) as sp, \
             tc.tile_pool(name="hp", bufs=hp_bufs) as hp, \
             tc.tile_pool(name="ob", bufs=ob_bufs) as ob, \
             tc.tile_pool(name="pg", bufs=bg, space="PSUM") as pg, \
             tc.tile_pool(name="ph", bufs=bh, space="PSUM") as ph, \
             tc.tile_pool(name="po", bufs=bo, space="PSUM") as po:

            # pending: previous pair awaiting staggered o emission.
            # [n, p0, pw, hbts, w_t, o_ps]; o_ps filled lazily at hb0.
            pending = [None]

            def o_mm(hb):
                # w2 contribution of hblock hb for the PREVIOUS pair, issued
                # during the current pair's hb slot: its mul finished a full
                # pair ago, so the PE's in-order stream never waits on DVE.
                pend = pending[0]
                if pend is None:
                    return
                n, p0, pw, hbts, w_t, o_ps = pend
                w2of = 2 * H
                if hb == 0:
                    for (s0, sw) in subs(pw):
                        o_ps[s0] = po.tile([128, 512], f32, tag="o",
                                           name=f"o{s0}")
                for (s0, sw) in subs(pw):
                    nc.tensor.matmul(
                        o_ps[s0][:, :sw],
                        w_t[:, w2of + hb * 128:w2of + (hb + 1) * 128],
                        hbts[hb][:, s0:s0 + sw],
                        start=(hb == 0), stop=(hb == HB - 1),
                    )

            def o_evac():
                pend = pending[0]
                if pend is None:
                    return
                n, p0, pw, hbts, w_t, o_ps = pend
                for (s0, sw) in subs(pw):
                    o_sb = ob.tile([128, 512], bf, tag="osb")
                    if copy_cnt[0] % evac_period in evac_dve:
                        nc.vector.tensor_copy(o_sb[:, :sw], o_ps[s0][:, :sw])
                    else:
                        nc.scalar.copy(o_sb[:, :sw], o_ps[s0][:, :sw])
                    copy_cnt[0] += 1
                    (nc.gpsimd if ydma_gp else nc.sync).dma_start(
                        yT[n][:, p0 + s0:p0 + s0 + sw], o_sb[:, :sw]
                    )
                pending[0] = None

            def flush_pending():
                for hb in range(HB):
                    o_mm(hb)
                o_evac()

            const_hbt = [None]

            def head_pe_only(n, x_t, w_t):
                # diagnostic: the exact PE instruction stream (g/h1/o matmuls
                # with the same weight-change pattern) but no ACT/DVE deps.
                # diag="pe_silu": adds the real silu chain (PE waits the g
                # ring like the real kernel) but still no DVE muls.
                w1of, wgof, w2of = 0, H, 2 * H
                hbt = const_hbt[0]
                for (p0, pw) in pairs:
                    o_ps = {}
                    for (s0, sw) in subs(pw):
                        o_ps[s0] = po.tile([128, 512], f32, tag="o",
                                           name=f"o{s0}")
                    for hb in range(HB):
                        g_t = pg.tile([128, gW], f32, tag="g")
                        for (s0, sw) in subs(pw):
                            nc.tensor.matmul(
                                g_t[:, s0:s0 + sw],
                                w_t[:, wgof + hb * 128:wgof + (hb + 1) * 128],
                                x_t[:, p0 + s0:p0 + s0 + sw],
                                start=True, stop=True,
                            )
                        if diag == "pe_silu":
                            sil = sp.tile([128, gW], f32, tag="sil")
                            nc.scalar.activation(sil[:, :pw], g_t[:, :pw],
                                                 SILU)
                        for (s0, sw) in subs(pw):
                            h1_t = ph.tile([128, 512], f32, tag="h1")
                            nc.tensor.matmul(
                                h1_t[:, :sw],
                                w_t[:, w1of + hb * 128:w1of + (hb + 1) * 128],
                                x_t[:, p0 + s0:p0 + s0 + sw],
                                start=True, stop=True,
                            )
                        for (s0, sw) in subs(pw):
                            nc.tensor.matmul(
                                o_ps[s0][:, :sw],
                                w_t[:, w2of + hb * 128:w2of + (hb + 1) * 128],
                                hbt[:, s0:s0 + sw],
                                start=(hb == 0), stop=(hb == HB - 1),
                            )

            def head(n, x_t, w_t):
                if diag in ("pe_only", "pe_silu"):
                    return head_pe_only(n, x_t, w_t)
                w1of, wgof = 0, H
                # head 0 starts with the small tail pair: its g-matmul only
                # needs a tiny x slice, so silu/mul start ~1.5us earlier.
                hpairs = (pairs[-1:] + pairs[:-1]) if n == 0 else pairs
                for (p0, pw) in hpairs:
                    hbts = []
                    for hb in range(HB):
                        g_t = pg.tile([128, gW], f32, tag="g")
                        for (s0, sw) in subs(pw):
                            nc.tensor.matmul(
                                g_t[:, s0:s0 + sw],
                                w_t[:, wgof + hb * 128:wgof + (hb + 1) * 128],
                                x_t[:, p0 + s0:p0 + s0 + sw],
                                start=True, stop=True,
                            )
                        sil = sp.tile([128, gW], f32, tag="sil")
                        if diag in ("slim_silu", "skeleton"):
                            # diagnostic: tiny silu keeps the dep, drops load
                            nc.vector.memset(sil[:, :pw], 1.0)
                            nc.scalar.activation(sil[:, :64], g_t[:, :64], SILU)
                        elif silu_split:
                            # per-512 silu: +~144ns/hb ACT but the first mul
                            # can start ~500ns earlier (finer ACT->DVE pipe)
                            for (s0, sw) in subs(pw):
                                nc.scalar.activation(sil[:, s0:s0 + sw],
                                                     g_t[:, s0:s0 + sw], SILU)
                        else:
                            nc.scalar.activation(sil[:, :pw], g_t[:, :pw], SILU)
                        hbt = hp.tile([128, gW], bf, tag=f"hbt{hb}")
                        if diag in ("slim_mul", "skeleton"):
                            nc.vector.memset(hbt[:, :pw], 1.0)
                        for (s0, sw) in subs(pw):
                            h1_t = ph.tile([128, 512], f32, tag="h1")
                            nc.tensor.matmul(
                                h1_t[:, :sw],
                                w_t[:, w1of + hb * 128:w1of + (hb + 1) * 128],
                                x_t[:, p0 + s0:p0 + s0 + sw],
                                start=True, stop=True,
                            )
                            if diag in ("slim_mul", "skeleton"):
                                if s0 == 0:
                                    nc.vector.tensor_mul(
                                        hbt[:, :64], h1_t[:, :64], sil[:, :64]
                                    )
                            else:
                                nc.vector.tensor_mul(
                                    hbt[:, s0:s0 + sw], h1_t[:, :sw],
                                    sil[:, s0:s0 + sw]
                                )
                        hbts.append(hbt)
                        if pipe:
                            o_mm(hb)
                    if pipe:
                        o_evac()
                        pending[0] = [n, p0, pw, hbts, w_t, {}]
                    else:
                        pending[0] = [n, p0, pw, hbts, w_t, {}]
                        flush_pending()

            def body():
                if warm_act:
                    warm = sp.tile([128, 1], f32, tag="warm", name="warm")
                    nc.vector.memset(warm[:], 0.0)
                    nc.scalar.activation(warm[:], warm[:], SILU)
                if diag in ("pe_only", "pe_silu") and const_hbt[0] is None:
                    cb = hp.tile([128, gW], bf, tag="chbt", name="chbt")
                    nc.vector.memset(cb[:], 0.5)
                    const_hbt[0] = cb
                for n in range(N):
                    # w first: the first g-matmul needs the weights; x can
                    # stream in behind it (head 0 consumes the tail slice
                    # first, so that chunk is DMA'd first).
                    w_t = wp.tile([128, WCOL], bf, tag="w")
                    nc.sync.dma_start(w_t[:], wall[n])
                    x_t = xp.tile([128, C], bf, tag="x")
                    if split_x_dma and n == 0:
                        for (p0, pw) in (pairs[-1:] + pairs[:-1]):
                            nc.sync.dma_start(x_t[:, p0:p0 + pw],
                                              xT[n][:, p0:p0 + pw])
                    else:
                        nc.sync.dma_start(x_t[:], xT[n])
                    head(n, x_t, w_t)
                if pipe and pending[0] is not None:
                    flush_pending()

            if reps == 1:
                body()
            else:
                with tc.For_i(0, reps, 1):
                    body()
    nc.finalize()
    return nc


def _route(x, router_w):
    import jax
    import jax.numpy as jnp

    router_logits = jnp.asarray(x).reshape(B, N * D) @ jnp.asarray(router_w).T
    topk_logits, topk_idx = jax.lax.top_k(router_logits, 2)
    topk_w = jax.nn.softmax(topk_logits, axis=-1)
    return np.asarray(topk_idx), np.asarray(topk_w).astype(np.float32)


def _dispatch(x, topk_idx, topk_w):
    idx_list, wgt_list = [], []
    for e in range(E):
        sel = np.nonzero((topk_idx == e).any(axis=1))[0]
        we = np.where(topk_idx[sel, 0] == e, topk_w[sel, 0], topk_w[sel, 1])
        idx_list.append(sel)
        wgt_list.append(we.astype(np.float32))
    maxL = max(max(len(s) for s in idx_list), 1)
    C = ((maxL + CAP_ALIGN - 1) // CAP_ALIGN) * CAP_ALIGN
    chunks = []
    c0 = 0
    while c0 < C:
        cw = 512 if C - c0 >= 512 else C - c0
        chunks.append((c0, cw))
        c0 += cw
    return idx_list, wgt_list, C, tuple(chunks)


def _make_in_maps(x, w1, w_gate, w2, idx_list, wgt_list, C, bf16=USE_BF16):
    if bf16:
        import ml_dtypes
        dt = ml_dtypes.bfloat16
    else:
        dt = np.float32
    in_maps = []
    xTfull = np.ascontiguousarray(x.transpose(1, 2, 0).astype(dt))  # (N,128,B)
    for e in range(E):
        sel = idx_list[e]
        L = len(sel)
        xg = np.zeros((N, 128, C), dt)
        if L:
            xg[:, :, :L] = xTfull[:, :, sel]
        w2r = w2[e].reshape(N, HB, 128, 128).transpose(0, 2, 1, 3).reshape(N, 128, H)
        wcat = np.ascontiguousarray(np.concatenate(
            [w1[e].astype(dt), w_gate[e].astype(dt), w2r.astype(dt)], axis=2
        ))  # (N,128,3H)
        in_maps.append({"xT": xg, "wall": wcat})
    return in_maps


_runner_cache = {}


def _make_runner(nc):
    """Cached jitted executor equivalent to bass2jax.run_bass_via_pjrt,
    avoiding per-call retrace/rejit of the shard_map wrapper."""
    import jax
    import concourse.mybir as mybir
    from concourse import bass2jax
    from jax.sharding import Mesh, PartitionSpec
    from jax.experimental.shard_map import shard_map

    bass2jax.install_neuronx_cc_hook()
    partition_name = nc.partition_id_tensor.name if nc.partition_id_tensor else None
    in_names, out_names, out_avals, out_shapes = [], [], [], []
    for alloc in nc.m.functions[0].allocations:
        if not isinstance(alloc, mybir.MemoryLocationSet):
            continue
        name = alloc.memorylocations[0].name
        if alloc.kind == "ExternalInput":
            if name != partition_name:
                in_names.append(name)
        elif alloc.kind == "ExternalOutput":
            shape = tuple(alloc.tensor_shape)
            dtype = mybir.dt.np(alloc.dtype)
            out_names.append(name)
            out_avals.append(jax.core.ShapedArray(shape, dtype))
            out_shapes.append((shape, dtype))
    all_in_names = list(in_names) + list(out_names)
    if partition_name is not None:
        all_in_names.append(partition_name)

    def _body(*args):
        operands = list(args)
        if partition_name is not None:
            operands.append(bass2jax.partition_id_tensor())
        return tuple(bass2jax._bass_exec_p.bind(
            *operands,
            out_avals=tuple(out_avals),
            in_names=tuple(all_in_names),
            out_names=tuple(out_names),
            lowering_input_output_aliases=(),
            sim_require_finite=True,
            sim_require_nnan=True,
            nc=nc,
        ))

    mesh = Mesh(np.asarray(jax.devices()[:N_CORES]), ("core",))
    nio = len(in_names) + len(out_names)
    sharded = jax.jit(
        shard_map(_body, mesh=mesh,
                  in_specs=(PartitionSpec("core"),) * nio,
                  out_specs=(PartitionSpec("core"),) * len(out_names),
                  check_rep=False),
        keep_unused=True,
    )

    def run(in_maps):
        concat_in = [
            np.concatenate([np.asarray(in_maps[c][nm]) for c in range(N_CORES)],
                           axis=0)
            for nm in in_names
        ]
        concat_zeros = [
            np.zeros((N_CORES * s[0], *s[1:]), d) for (s, d) in out_shapes
        ]
        outs = sharded(*(concat_in + concat_zeros))
        outs = [np.asarray(o) for o in outs]
        results = []
        for c in range(N_CORES):
            res = {}
            for (nm, o, (s, d)) in zip(out_names, outs, out_shapes):
                res[nm] = o[c * s[0]:(c + 1) * s[0]]
            results.append(res)
        return results

    return run


def kernel(**inputs):
    x = np.asarray(inputs["x"], dtype=np.float32)
    router_w = np.asarray(inputs["router_w"], dtype=np.float32)
    w1 = np.asarray(inputs["w1"], dtype=np.float32)
    w_gate = np.asarray(inputs["w_gate"], dtype=np.float32)
    w2 = np.asarray(inputs["w2"], dtype=np.float32)

    topk_idx, topk_w = _route(x, router_w)
    idx_list, wgt_list, C, chunks = _dispatch(x, topk_idx, topk_w)

    key = (C, chunks, 1, USE_BF16)
    if key not in _nc_cache:
        _nc_cache[key] = _build_bass(C, chunks)
    nc = _nc_cache[key]

    in_maps = _make_in_maps(x, w1, w_gate, w2, idx_list, wgt_list, C)

    if key not in _runner_cache:
        from concourse import bass_utils
        res = bass_utils.run_bass_kernel_spmd(
            nc, in_maps, core_ids=list(range(N_CORES)), trace=False
        )
        results = res.results
        _runner_cache[key] = _make_runner(nc)
    else:
        results = _runner_cache[key](in_maps)

    out = np.zeros((B, N, D), np.float32)
    for e in range(E):
        sel = idx_list[e]
        L = len(sel)
        if L:
            yT = np.asarray(results[e]["yT"], dtype=np.float32)  # (N,128,C)
            out[sel] += yT[:, :, :L].transpose(2, 0, 1) * \
                wgt_list[e][:, None, None]
    return out
